# revision 1
# baseline (speedup 1.0000x reference)
"""Trainium2 Bass kernel for nn_MultiHeadAttention (B=4, S=2048, D=1024, H=16, causal).

Sharding: 8 cores = 4 batches x 2 head-halves (8 heads each). Every core runs an
identical SPMD program: Q/K/V projections for its 8 heads over its batch's 2048
tokens, causal flash-attention, and a partial output projection over its 512
head-dims. Host unshard adds the two partial outputs per batch (+ bo).

All matmuls run as float32r (fp32 storage, reduced-precision multiply, full PE
rate at moving-dim >= 256) accumulating into fp32 PSUM.
"""

import os
import sys

for _p in ("/opt/trn_rl_repo", "/root/.axon_site/_ro/trn_rl_repo"):
    if os.path.isdir(_p) and _p not in sys.path:
        sys.path.insert(0, _p)

import numpy as np

B, S, D, H = 4, 2048, 1024, 16
HD = D // H  # 64
DH = D // 2  # 512 dims per head-half
NCORES = 8
QT_TILES = 4      # 512-token q tiles
PAIRS = 4         # head pairs per core (8 heads)
ICHUNKS = 8       # 128-row feature chunks of D
TT16 = 16         # 128-token tiles


def _build_nc(repeat=1):
    import concourse.mybir as mybir
    import concourse.tile as tile
    from concourse import bacc

    F32 = mybir.dt.float32
    F32R = mybir.dt.float32r
    BF16 = mybir.dt.bfloat16
    ACTF = mybir.ActivationFunctionType
    ALU = mybir.AluOpType

    nc = bacc.Bacc("TRN2", target_bir_lowering=False, debug=False, num_devices=NCORES)

    xqT = nc.dram_tensor("xqT", [D, S], F32R, kind="ExternalInput")
    xkT = nc.dram_tensor("xkT", [D, S], F32R, kind="ExternalInput")
    xvT = nc.dram_tensor("xvT", [D, S], F32R, kind="ExternalInput")
    wqt = nc.dram_tensor("wqt", [D, DH], F32R, kind="ExternalInput")
    wkt = nc.dram_tensor("wkt", [D, DH], F32R, kind="ExternalInput")
    wvt = nc.dram_tensor("wvt", [D, DH], F32R, kind="ExternalInput")
    wot = nc.dram_tensor("wot", [DH, D], F32R, kind="ExternalInput")
    bq = nc.dram_tensor("bq", [DH], F32, kind="ExternalInput")
    bk = nc.dram_tensor("bk", [DH], F32, kind="ExternalInput")
    bv = nc.dram_tensor("bv", [1, DH], F32R, kind="ExternalInput")
    onesd = nc.dram_tensor("ones", [128, 128], F32R, kind="ExternalInput")
    onesbd = nc.dram_tensor("onesb", [128, 64], BF16, kind="ExternalInput")
    masksd = nc.dram_tensor("masks", [4, 128, 1024], BF16, kind="ExternalInput")
    outp = nc.dram_tensor("outp", [S, D], F32, kind="ExternalOutput")

    with tile.TileContext(nc) as tc:
        with (
            tc.tile_pool(name="const", bufs=1) as cp,
            tc.tile_pool(name="persist", bufs=1) as pp,
        ):
            ones_t = cp.tile([128, 128], F32R, tag="ones", name="ones_t")
            bv_t = cp.tile([1, DH], F32R, tag="bv", name="bv_t")
            nc.sync.dma_start(ones_t[:], onesd.ap())
            onesb_t = cp.tile([128, 64], BF16, tag="onesb", name="onesb_t")
            nc.sync.dma_start(onesb_t[:], onesbd.ap())
            nc.sync.dma_start(bv_t[:], bv.ap())
            mask_t = []
            for m in range(4):
                mt = cp.tile([128, 1024], BF16, tag=f"mask{m}", name=f"mask_t{m}")
                nc.sync.dma_start(mt[:], masksd.ap()[m])
                mask_t.append(mt)
            bq_t, bk_t = [], []
            for p in range(PAIRS):
                t1 = cp.tile([128, 1], F32, tag=f"bq{p}", name=f"bq_t{p}")
                nc.sync.dma_start(t1[:], bq.ap()[128 * p : 128 * (p + 1)])
                bq_t.append(t1)
                t2 = cp.tile([128, 1], F32, tag=f"bk{p}", name=f"bk_t{p}")
                nc.sync.dma_start(t2[:], bk.ap()[128 * p : 128 * (p + 1)])
                bk_t.append(t2)
            # wo resident: 4 chunk tiles [128 d, 1024 e]
            wo_t = []
            for p in range(PAIRS):
                wt = pp.tile([128, D], F32R, tag=f"wo{p}", name=f"wo_t{p}")
                nc.sync.dma_start(wt[:], wot.ap()[128 * p : 128 * (p + 1), :])
                wo_t.append(wt)

            # persistent activations (feature-major)
            QT = [pp.tile([128, S], F32R, tag=f"qt{p}", name=f"QT{p}") for p in range(PAIRS)]
            KT = [pp.tile([128, S], F32R, tag=f"kt{p}", name=f"KT{p}") for p in range(PAIRS)]
            V = [pp.tile([128, DH], BF16, tag=f"v{i}", name=f"V{i}") for i in range(TT16)]

            # ---------------- projection phases ----------------
            for _rep in range(repeat):
              with (
                  tc.tile_pool(name="xtb", bufs=16) as xtbp,
                  tc.tile_pool(name="wp", bufs=16) as wp,
                  tc.tile_pool(name="pps", bufs=8, space="PSUM") as projps,
              ):
                  def qk_proj(xinT, wdram, dst, bias_tiles):
                      w_t = []
                      for c in range(ICHUNKS):
                          wt = wp.tile([128, DH], F32R, tag="w", name="w_t")
                          nc.sync.dma_start(wt[:], wdram.ap()[128 * c : 128 * (c + 1), :])
                          w_t.append(wt)
                      for t in range(QT_TILES):
                          tsl = slice(512 * t, 512 * (t + 1))
                          pps_t = [projps.tile([128, 512], F32, tag="pp", name="pp_tile") for _ in range(PAIRS)]
                          for c in range(ICHUNKS):
                              xtb = xtbp.tile([128, 512], F32R, tag="xtb", name="xtb_tile")
                              nc.sync.dma_start(xtb[:], xinT.ap()[128 * c : 128 * (c + 1), tsl])
                              for p in range(PAIRS):
                                  nc.tensor.matmul(
                                      pps_t[p][:],
                                      w_t[c][:, 128 * p : 128 * (p + 1)],
                                      xtb[:],
                                      start=(c == 0),
                                      stop=(c == ICHUNKS - 1),
                                  )
                          for p in range(PAIRS):
                              nc.scalar.activation(
                                  dst[p][:, 512 * t : 512 * (t + 1)],
                                  pps_t[p][:],
                                  ACTF.Identity,
                                  bias=bias_tiles[p][:],
                              )

                  qk_proj(xqT, wqt, QT, bq_t)
                  qk_proj(xkT, wkt, KT, bk_t)

                  # V projection: token-major [128 t, 512 o]
                  wv_t = []
                  for c in range(ICHUNKS):
                      wt = wp.tile([128, DH], F32R, tag="w", name="w_t")
                      nc.sync.dma_start(wt[:], wvt.ap()[128 * c : 128 * (c + 1), :])
                      wv_t.append(wt)
                  # bv broadcast tile [128, 512] built once
                  pbv = projps.tile([128, 512], F32, tag="pp", name="pbv_tile")
                  nc.tensor.matmul(pbv[:], ones_t[0:1, 0:128], bv_t[:], start=True, stop=True)
                  bvb = wp.tile([128, DH], F32R, tag="bvb", name="bvb_tile")
                  nc.vector.tensor_copy(bvb[:], pbv[:])
                  for t in range(QT_TILES):
                      tsl = slice(512 * t, 512 * (t + 1))
                      pv_t = [projps.tile([128, 512], F32, tag="pp", name="pv_tile") for _ in range(4)]
                      for c in range(ICHUNKS):
                          xtb = xtbp.tile([128, 512], F32R, tag="xtb", name="xtb_tile")
                          nc.sync.dma_start(xtb[:], xvT.ap()[128 * c : 128 * (c + 1), tsl])
                          for b in range(4):
                              nc.tensor.matmul(
                                  pv_t[b][:],
                                  xtb[:, 128 * b : 128 * (b + 1)],
                                  wv_t[c][:],
                                  start=(c == 0),
                                  stop=(c == ICHUNKS - 1),
                              )
                      for b in range(4):
                          # V = proj + bv (broadcast add fused into the eviction)
                          nc.vector.scalar_tensor_tensor(
                              V[4 * t + b][:], pv_t[b][:], 1.0, bvb[:],
                              ALU.mult, ALU.add,
                          )

              # ---------------- attention + out-projection ----------------
              with (
                  tc.tile_pool(name="ppool", bufs=6) as ppool,
                  tc.tile_pool(name="rpool", bufs=3) as rpool,
                  tc.tile_pool(name="apool", bufs=3) as apool,
                  tc.tile_pool(name="osb", bufs=4) as osbp,
                  tc.tile_pool(name="sps", bufs=2, space="PSUM") as sps,
                  tc.tile_pool(name="acc", bufs=1, space="PSUM") as accps,
                  tc.tile_pool(name="ops", bufs=2, space="PSUM") as outps,
              ):
                  for t in range(QT_TILES):
                      qsl = slice(512 * t, 512 * (t + 1))
                      nch = 4 * (t + 1)
                      A = []
                      for p in range(PAIRS):
                          psO = accps.tile([128, 512], F32, tag="o", name="psO_t")
                          psSum = accps.tile([128, 512], F32, tag="sum", name="psSum_t")
                          for j in range(nch):
                              ksl = slice(128 * j, 128 * (j + 1))
                              s01 = sps.tile([128, 1024], F32, tag="s01", name="s01_t")
                              nc.tensor.matmul(
                                  s01[:, 0:512], KT[p][0:64, ksl], QT[p][0:64, qsl],
                                  start=True, stop=True,
                              )
                              nc.tensor.matmul(
                                  s01[:, 512:1024], KT[p][64:128, ksl], QT[p][64:128, qsl],
                                  start=True, stop=True,
                              )
                              pb = ppool.tile([128, 1024], BF16, tag="pb", name="pb_t")
                              nc.scalar.activation(pb[:], s01[:], ACTF.Exp, scale=0.125)
                              if j >= 4 * t:
                                  m = j - 4 * t
                                  nc.vector.scalar_tensor_tensor(
                                      pb[:], pb[:], 1.0, mask_t[m][:],
                                      ALU.mult, ALU.mult,
                                  )
                              p0 = pb[:, 0:512]
                              p1 = pb[:, 512:1024]
                              st = (j == 0)
                              sp = (j == nch - 1)
                              nc.tensor.matmul(
                                  psO[0:64, :], V[j][:, 128 * p : 128 * p + 64], p0,
                                  start=st, stop=sp,
                              )
                              nc.tensor.matmul(
                                  psO[64:128, :], V[j][:, 128 * p + 64 : 128 * (p + 1)], p1,
                                  start=st, stop=sp,
                              )
                              nc.tensor.matmul(
                                  psSum[0:64, :], onesb_t[:, 0:64], p0,
                                  start=st, stop=sp,
                              )
                              nc.tensor.matmul(
                                  psSum[64:128, :], onesb_t[:, 0:64], p1,
                                  start=st, stop=sp,
                              )
                          r = rpool.tile([128, 512], F32, tag="r", name="r_t")
                          nc.vector.reciprocal(r[:], psSum[:])
                          a = apool.tile([128, 512], F32R, tag=f"a{p}", name=f"a_t{p}")
                          nc.vector.scalar_tensor_tensor(
                              a[:], psO[:], 1.0, r[:], ALU.mult, ALU.mult
                          )
                          A.append(a)
                      # out-projection for this q tile (partial over our 512 dims)
                      for tl in range(4):
                          for eh in range(2):
                              po = outps.tile([128, 512], F32, tag="op", name="po_t")
                              for p in range(PAIRS):
                                  nc.tensor.matmul(
                                      po[:],
                                      A[p][:, 128 * tl : 128 * (tl + 1)],
                                      wo_t[p][:, 512 * eh : 512 * (eh + 1)],
                                      start=(p == 0),
                                      stop=(p == PAIRS - 1),
                                  )
                              ob = osbp.tile([128, 512], F32, tag="ob", name="ob_t")
                              if (tl + eh) % 2 == 0:
                                  nc.vector.tensor_copy(ob[:], po[:])
                              else:
                                  nc.scalar.activation(ob[:], po[:], ACTF.Copy)
                              r0 = 512 * t + 128 * tl
                              nc.sync.dma_start(
                                  outp.ap()[r0 : r0 + 128, 512 * eh : 512 * (eh + 1)],
                                  ob[:],
                              )

    nc.compile()
    return nc


_RT = {}


def _get_runtime():
    if "rt" in _RT:
        return _RT["rt"]

    import jax
    import numpy as np
    from jax.experimental.shard_map import shard_map
    from jax.sharding import Mesh, PartitionSpec

    import concourse.mybir as mybir
    from concourse.bass2jax import (
        _bass_exec_p,
        install_neuronx_cc_hook,
        partition_id_tensor,
    )

    nc = _build_nc()
    install_neuronx_cc_hook()

    partition_name = nc.partition_id_tensor.name if nc.partition_id_tensor else None
    in_names, out_names, out_avals, zero_shapes = [], [], [], []
    for alloc in nc.m.functions[0].allocations:
        if not isinstance(alloc, mybir.MemoryLocationSet):
            continue
        if not alloc.memorylocations:
            continue
        name = alloc.memorylocations[0].name
        if alloc.kind == "ExternalInput":
            if name != partition_name:
                in_names.append(name)
        elif alloc.kind == "ExternalOutput":
            shape = tuple(alloc.tensor_shape)
            dtype = mybir.dt.np(alloc.dtype)
            out_names.append(name)
            out_avals.append(jax.core.ShapedArray(shape, dtype))
            zero_shapes.append((shape, dtype))
    n_params = len(in_names)
    n_outs = len(out_names)
    all_in_names = list(in_names) + list(out_names)
    if partition_name is not None:
        all_in_names.append(partition_name)
    donate = tuple(range(n_params, n_params + n_outs))

    def _body(*args):
        operands = list(args)
        if partition_name is not None:
            operands.append(partition_id_tensor())
        outs = _bass_exec_p.bind(
            *operands,
            out_avals=tuple(out_avals),
            in_names=tuple(all_in_names),
            out_names=tuple(out_names),
            lowering_input_output_aliases=(),
            sim_require_finite=True,
            sim_require_nnan=True,
            nc=nc,
        )
        return tuple(outs)

    devices = jax.devices()[:NCORES]
    assert len(devices) == NCORES
    mesh = Mesh(np.asarray(devices), ("core",))
    in_specs = (PartitionSpec("core"),) * (n_params + n_outs)
    out_specs = (PartitionSpec("core"),) * n_outs
    fn = jax.jit(
        shard_map(_body, mesh=mesh, in_specs=in_specs, out_specs=out_specs,
                  check_rep=False),
        donate_argnums=donate,
        keep_unused=True,
    )
    rt = {
        "fn": fn,
        "in_names": in_names,
        "out_names": out_names,
        "zero_shapes": zero_shapes,
        "n_params": n_params,
        "mesh": mesh,
        "nc": nc,
    }
    _RT["rt"] = rt
    return rt


def _make_masks():
    kk = np.arange(128, dtype=np.int64)[:, None]
    q = np.arange(512, dtype=np.int64)[None, :]
    masks = np.zeros((4, 128, 1024), dtype=np.float32)
    for m in range(4):
        half = ((128 * m + kk) <= q).astype(np.float32)
        masks[m, :, 0:512] = half
        masks[m, :, 512:1024] = half
    return masks


def _shard_inputs(query, key, value, Wq, bq, Wk, bk, Wv, bv, Wo, bo, pad_mask):
    f = np.float32
    query = np.asarray(query, f).reshape(B, S, D)
    key = np.asarray(key, f).reshape(B, S, D)
    value = np.asarray(value, f).reshape(B, S, D)
    import ml_dtypes
    bf = ml_dtypes.bfloat16
    consts = {
        "ones": np.ones((128, 128), f),
        "onesb": np.ones((128, 64), bf),
        "masks": _make_masks().astype(bf),
    }
    xT = {b: {
        "xqT": np.ascontiguousarray(query[b].T),
        "xkT": np.ascontiguousarray(key[b].T),
        "xvT": np.ascontiguousarray(value[b].T),
    } for b in range(B)}
    wT = {
        "q": np.asarray(Wq, f).T.copy(),
        "k": np.asarray(Wk, f).T.copy(),
        "v": np.asarray(Wv, f).T.copy(),
        "o": np.asarray(Wo, f).T.copy(),
    }
    in_maps = []
    for c in range(NCORES):
        b = c // 2
        hh = c % 2
        sl = slice(DH * hh, DH * (hh + 1))
        m = {
            **xT[b],
            "wqt": np.ascontiguousarray(wT["q"][:, sl]),
            "wkt": np.ascontiguousarray(wT["k"][:, sl]),
            "wvt": np.ascontiguousarray(wT["v"][:, sl]),
            "wot": np.ascontiguousarray(wT["o"][sl, :]),
            "bq": np.ascontiguousarray(np.asarray(bq, f)[sl]),
            "bk": np.ascontiguousarray(np.asarray(bk, f)[sl]),
            "bv": np.ascontiguousarray(np.asarray(bv, f)[sl]).reshape(1, DH),
            **consts,
        }
        in_maps.append(m)
    return in_maps


def _run(rt, in_maps):
    import jax
    import numpy as np

    n = rt["n_params"]
    concat_in = [
        np.concatenate([np.asarray(in_maps[c][name]) for c in range(NCORES)], axis=0)
        for name in rt["in_names"]
    ]
    concat_zeros = [
        np.zeros((NCORES * sh[0], *sh[1:]), dt) for sh, dt in rt["zero_shapes"]
    ]
    out_arrs = rt["fn"](*concat_in, *concat_zeros)
    res = []
    for c in range(NCORES):
        d = {}
        for i, name in enumerate(rt["out_names"]):
            sh = rt["zero_shapes"][i][0]
            d[name] = np.asarray(out_arrs[i]).reshape(NCORES, *sh)[c]
        res.append(d)
    return res


def kernel(**inputs):
    rt = _get_runtime()
    in_maps = _shard_inputs(**inputs)
    res = _run(rt, in_maps)
    bo = np.asarray(inputs["bo"], np.float32)
    out = np.empty((B, S, D), dtype=np.float32)
    for b in range(B):
        out[b] = res[2 * b]["outp"] + res[2 * b + 1]["outp"] + bo
    return out



# revision 7
# speedup vs baseline: 1.2030x; 1.2030x over previous
"""Trainium2 Bass kernel for nn_MultiHeadAttention (B=4, S=2048, D=1024, H=16, causal).

Sharding: 8 cores = 4 batches x 2 head-halves (8 heads each). Every core runs an
identical SPMD program: Q/K/V projections for its 8 heads over its batch's 2048
tokens, causal flash-attention, and a partial output projection over its 512
head-dims. Host unshard adds the two partial outputs per batch (+ bo).

v2: all-bf16 operands (fp32 PSUM accumulation), softmax denominator fused into
the PV matmul via a ones-column appended to each head's V block (V tiles are
[128 tok, 8 heads, 65] with col 64 of each head group = 1.0), denominator
broadcast via a tiny [2,128]x[2,512] matmul, causal mask via tensor_tensor.
"""

import os
import sys

for _p in ("/opt/trn_rl_repo", "/root/.axon_site/_ro/trn_rl_repo"):
    if os.path.isdir(_p) and _p not in sys.path:
        sys.path.insert(0, _p)

import numpy as np

B, S, D, H = 4, 2048, 1024, 16
HD = D // H  # 64
DH = D // 2  # 512 dims per head-half
NCORES = 8
QT_TILES = 4      # 512-token q tiles
PAIRS = 4         # head pairs per core (8 heads)
ICHUNKS = 8       # 128-row feature chunks of D
TT16 = 16         # 128-token tiles


def _build_nc(repeat=1):
    import concourse.mybir as mybir
    import concourse.tile as tile
    from concourse import bacc

    F32 = mybir.dt.float32
    F32R = mybir.dt.float32r
    BF16 = mybir.dt.bfloat16
    ACTF = mybir.ActivationFunctionType
    ALU = mybir.AluOpType

    nc = bacc.Bacc("TRN2", target_bir_lowering=False, debug=False, num_devices=NCORES)

    xqT = nc.dram_tensor("xqT", [D, S], BF16, kind="ExternalInput")
    xkT = nc.dram_tensor("xkT", [D, S], BF16, kind="ExternalInput")
    xvT = nc.dram_tensor("xvT", [D, S], BF16, kind="ExternalInput")
    wqt = nc.dram_tensor("wqt", [D, DH], BF16, kind="ExternalInput")
    wkt = nc.dram_tensor("wkt", [D, DH], BF16, kind="ExternalInput")
    wvt = nc.dram_tensor("wvt", [D, DH], BF16, kind="ExternalInput")
    wot = nc.dram_tensor("wot", [DH, D], BF16, kind="ExternalInput")
    bq = nc.dram_tensor("bq", [DH], F32, kind="ExternalInput")
    bk = nc.dram_tensor("bk", [DH], F32, kind="ExternalInput")
    bv = nc.dram_tensor("bv", [1, DH], BF16, kind="ExternalInput")
    onescol = nc.dram_tensor("onescol", [1, 128], BF16, kind="ExternalInput")
    seld = nc.dram_tensor("selp", [65, 64], F32R, kind="ExternalInput")
    masksd = nc.dram_tensor("masks", [4, 128, 1024], BF16, kind="ExternalInput")
    outp = nc.dram_tensor("outp", [S, D], F32, kind="ExternalOutput")

    with tile.TileContext(nc) as tc:
        with (
            tc.tile_pool(name="const", bufs=1) as cp,
            tc.tile_pool(name="persist", bufs=1) as pp,
        ):
            onescol_t = cp.tile([1, 128], BF16, tag="onescol", name="onescol_t")
            nc.sync.dma_start(onescol_t[:], onescol.ap())
            selp_t = cp.tile([65, 64], F32R, tag="selp", name="selp_t")
            nc.sync.dma_start(selp_t[:], seld.ap())
            bv_t = cp.tile([1, DH], BF16, tag="bv", name="bv_t")
            nc.sync.dma_start(bv_t[:], bv.ap())
            mask_t = []
            for m in range(4):
                mt = cp.tile([128, 1024], BF16, tag=f"mask{m}", name=f"mask_t{m}")
                nc.sync.dma_start(mt[:], masksd.ap()[m])
                mask_t.append(mt)
            bq_t, bk_t = [], []
            for p in range(PAIRS):
                t1 = cp.tile([128, 1], F32, tag=f"bq{p}", name=f"bq_t{p}")
                nc.sync.dma_start(t1[:], bq.ap()[128 * p : 128 * (p + 1)])
                bq_t.append(t1)
                t2 = cp.tile([128, 1], F32, tag=f"bk{p}", name=f"bk_t{p}")
                nc.sync.dma_start(t2[:], bk.ap()[128 * p : 128 * (p + 1)])
                bk_t.append(t2)
            # wo resident: 4 chunk tiles [128 d, 1024 e]
            wo_t = []
            for p in range(PAIRS):
                wt = pp.tile([128, D], BF16, tag=f"wo{p}", name=f"wo_t{p}")
                nc.sync.dma_start(wt[:], wot.ap()[128 * p : 128 * (p + 1), :])
                wo_t.append(wt)

            # persistent activations (feature-major Q/K; token-major V)
            QT = [pp.tile([128, S], BF16, tag=f"qt{p}", name=f"QT{p}") for p in range(PAIRS)]
            KT = [pp.tile([128, S], BF16, tag=f"kt{p}", name=f"KT{p}") for p in range(PAIRS)]
            # V: [128 tok, 8 head-groups, 65]; col 64 of each group stays 1.0
            V = [pp.tile([128, 8, 96], BF16, tag=f"v{i}", name=f"V{i}") for i in range(TT16)]
            for i in range(TT16):
                nc.vector.memset(V[i][:, :, 64:96], 0.0)
                nc.vector.memset(V[i][:, :, 64:65], 1.0)

            for _rep in range(repeat):
              # ---------------- projection phase ----------------
              with (
                  tc.tile_pool(name="xtb", bufs=16) as xtbp,
                  tc.tile_pool(name="wp", bufs=26) as wp,
                  tc.tile_pool(name="pps", bufs=2, space="PSUM") as projps,
              ):
                  # bv broadcast tile [128, 512] built once per rep
                  pbv = projps.tile([128, 512], F32, tag="pp", name="pbv_tile")
                  nc.tensor.matmul(pbv[:], onescol_t[:], bv_t[:], start=True, stop=True)
                  bvb = wp.tile([128, DH], BF16, tag="bvb", name="bvb_tile")
                  nc.vector.tensor_copy(bvb[:], pbv[:])

                  def load_w(wdram):
                      w_t = []
                      for c in range(ICHUNKS):
                          wt = wp.tile([128, DH], BF16, tag="w", name="w_t")
                          nc.sync.dma_start(wt[:], wdram.ap()[128 * c : 128 * (c + 1), :])
                          w_t.append(wt)
                      return w_t

                  def qk_proj(xinT, w_t, dst, bias_tiles):
                      for t in range(QT_TILES):
                          tsl = slice(512 * t, 512 * (t + 1))
                          xtb = []
                          for c in range(ICHUNKS):
                              xt = xtbp.tile([128, 512], BF16, tag="xtb", name="xtb_tile")
                              nc.sync.dma_start(xt[:], xinT.ap()[128 * c : 128 * (c + 1), tsl])
                              xtb.append(xt)
                          for p in range(PAIRS):
                              pps_t = projps.tile([128, 512], F32, tag="pp", name="pp_tile")
                              for c in range(ICHUNKS):
                                  nc.tensor.matmul(
                                      pps_t[:],
                                      w_t[c][:, 128 * p : 128 * (p + 1)],
                                      xtb[c][:],
                                      start=(c == 0),
                                      stop=(c == ICHUNKS - 1),
                                  )
                              nc.scalar.activation(
                                  dst[p][:, tsl], pps_t[:], ACTF.Identity,
                                  bias=bias_tiles[p][:],
                              )

                      return

                  wq_t = load_w(wqt)
                  qk_proj(xqT, wq_t, QT, bq_t)
                  wk_t = load_w(wkt)
                  qk_proj(xkT, wk_t, KT, bk_t)

                  # V projection: token-major [128 t, 512 o] -> strided into V tiles
                  wv_t = load_w(wvt)
                  for t in range(QT_TILES):
                      tsl = slice(512 * t, 512 * (t + 1))
                      xtb = []
                      for c in range(ICHUNKS):
                          xt = xtbp.tile([128, 512], BF16, tag="xtb", name="xtb_tile")
                          nc.sync.dma_start(xt[:], xvT.ap()[128 * c : 128 * (c + 1), tsl])
                          xtb.append(xt)
                      for b in range(4):
                          pv_t = projps.tile([128, 512], F32, tag="pp", name="pv_tile")
                          for c in range(ICHUNKS):
                              nc.tensor.matmul(
                                  pv_t[:],
                                  xtb[c][:, 128 * b : 128 * (b + 1)],
                                  wv_t[c][:],
                                  start=(c == 0),
                                  stop=(c == ICHUNKS - 1),
                              )
                          # V = proj + bv, written as 8 groups of 64 (skip ones cols)
                          nc.vector.scalar_tensor_tensor(
                              V[4 * t + b][:, :, 0:64], pv_t[:], 1.0, bvb[:],
                              ALU.mult, ALU.add,
                          )

              # ---------------- attention + out-projection ----------------
              with (
                  tc.tile_pool(name="pbp", bufs=4) as pbp,
                  tc.tile_pool(name="rpool", bufs=3) as rpool,
                  tc.tile_pool(name="apool", bufs=2) as apool,
                  tc.tile_pool(name="osb", bufs=4) as osbp,
                  tc.tile_pool(name="sps", bufs=2, space="PSUM") as sps,
                  tc.tile_pool(name="acc", bufs=2, space="PSUM") as accps,
                  tc.tile_pool(name="shp", bufs=2, space="PSUM") as shps,
              ):
                  for t in range(QT_TILES):
                      qsl = slice(512 * t, 512 * (t + 1))
                      nch = 4 * (t + 1)
                      A = []
                      for p in range(PAIRS):
                          psA = accps.tile([96, 512], F32, tag="acc", name="psA_t")
                          psB = accps.tile([96, 512], F32, tag="acc", name="psB_t")
                          for j in range(nch):
                              ksl = slice(128 * j, 128 * (j + 1))
                              s01 = sps.tile([128, 1024], F32, tag="s01", name="s01_t")
                              nc.tensor.matmul(
                                  s01[:, 0:512], KT[p][0:64, ksl], QT[p][0:64, qsl],
                                  start=True, stop=True,
                              )
                              nc.tensor.matmul(
                                  s01[:, 512:1024], KT[p][64:128, ksl], QT[p][64:128, qsl],
                                  start=True, stop=True,
                              )
                              pb = pbp.tile([128, 1024], BF16, tag="pb", name="pb_t")
                              nc.scalar.activation(pb[:], s01[:], ACTF.Exp, scale=0.125)
                              if j >= 4 * t:
                                  m = j - 4 * t
                                  nc.vector.tensor_tensor(
                                      pb[:], pb[:], mask_t[m][:], ALU.mult
                                  )
                              st = (j == 0)
                              sp = (j == nch - 1)
                              nc.tensor.matmul(
                                  psA[:], V[j][:, 2 * p : 2 * p + 1, :], pb[:, 0:512],
                                  start=st, stop=sp,
                              )
                              nc.tensor.matmul(
                                  psB[:], V[j][:, 2 * p + 1 : 2 * p + 2, :], pb[:, 512:1024],
                                  start=st, stop=sp,
                              )
                          # softmax denominators -> reciprocal (aligned at row 64)
                          r2 = rpool.tile([65, 512], F32R, tag="r", name="r_t")
                          r2b = rpool.tile([65, 512], F32R, tag="r", name="r2b_t")
                          with nc.allow_low_precision(reason="f32r storage is fp32; only matmul multiply precision differs"):
                              nc.vector.reciprocal(r2[64:65, :], psA[64:65, :])
                              nc.vector.reciprocal(r2b[64:65, :], psB[64:65, :])
                          # broadcast each 1/sum to 64 partitions via K=1 matmuls
                          rbA = shps.tile([64, 512], F32, tag="sh", name="rbA_t")
                          rbB = shps.tile([64, 512], F32, tag="sh", name="rbB_t")
                          nc.tensor.matmul(rbA[:], selp_t[64:65, :], r2[64:65, :],
                                           start=True, stop=True)
                          nc.tensor.matmul(rbB[:], selp_t[64:65, :], r2b[64:65, :],
                                           start=True, stop=True)
                          rbc = rpool.tile([128, 512], BF16, tag="rbc", name="rbc_t")
                          nc.vector.tensor_copy(rbc[0:64, :], rbA[:])
                          nc.vector.tensor_scalar_mul(rbc[64:128, :], rbB[:], 1.0)
                          a = apool.tile([128, 512], BF16, tag=f"a{p}", name=f"a_t{p}")
                          nc.vector.scalar_tensor_tensor(
                              a[0:64, :], psA[0:64, :], 1.0, rbc[0:64, :],
                              ALU.mult, ALU.mult,
                          )
                          nc.vector.scalar_tensor_tensor(
                              a[64:128, :], psB[0:64, :], 1.0, rbc[64:128, :],
                              ALU.mult, ALU.mult,
                          )
                          A.append(a)
                      # out-projection for this q tile (partial over our 512 dims)
                      for tl in range(4):
                          for eh in range(2):
                              po = shps.tile([128, 512], F32, tag="sh", name="po_t")
                              for p in range(PAIRS):
                                  nc.tensor.matmul(
                                      po[:],
                                      A[p][:, 128 * tl : 128 * (tl + 1)],
                                      wo_t[p][:, 512 * eh : 512 * (eh + 1)],
                                      start=(p == 0),
                                      stop=(p == PAIRS - 1),
                                  )
                              ob = osbp.tile([128, 512], F32, tag="ob", name="ob_t")
                              if (tl + eh) % 2 == 0:
                                  nc.vector.tensor_copy(ob[:], po[:])
                              else:
                                  nc.scalar.activation(ob[:], po[:], ACTF.Copy)
                              r0 = 512 * t + 128 * tl
                              nc.sync.dma_start(
                                  outp.ap()[r0 : r0 + 128, 512 * eh : 512 * (eh + 1)],
                                  ob[:],
                              )

    nc.compile()
    return nc


_RT = {}


def _get_runtime():
    if "rt" in _RT:
        return _RT["rt"]

    import jax
    import numpy as np
    from jax.experimental.shard_map import shard_map
    from jax.sharding import Mesh, PartitionSpec

    import concourse.mybir as mybir
    from concourse.bass2jax import (
        _bass_exec_p,
        install_neuronx_cc_hook,
        partition_id_tensor,
    )

    nc = _build_nc()
    install_neuronx_cc_hook()

    partition_name = nc.partition_id_tensor.name if nc.partition_id_tensor else None
    in_names, out_names, out_avals, zero_shapes = [], [], [], []
    for alloc in nc.m.functions[0].allocations:
        if not isinstance(alloc, mybir.MemoryLocationSet):
            continue
        if not alloc.memorylocations:
            continue
        name = alloc.memorylocations[0].name
        if alloc.kind == "ExternalInput":
            if name != partition_name:
                in_names.append(name)
        elif alloc.kind == "ExternalOutput":
            shape = tuple(alloc.tensor_shape)
            dtype = mybir.dt.np(alloc.dtype)
            out_names.append(name)
            out_avals.append(jax.core.ShapedArray(shape, dtype))
            zero_shapes.append((shape, dtype))
    n_params = len(in_names)
    n_outs = len(out_names)
    all_in_names = list(in_names) + list(out_names)
    if partition_name is not None:
        all_in_names.append(partition_name)
    donate = tuple(range(n_params, n_params + n_outs))

    def _body(*args):
        operands = list(args)
        if partition_name is not None:
            operands.append(partition_id_tensor())
        outs = _bass_exec_p.bind(
            *operands,
            out_avals=tuple(out_avals),
            in_names=tuple(all_in_names),
            out_names=tuple(out_names),
            lowering_input_output_aliases=(),
            sim_require_finite=True,
            sim_require_nnan=True,
            nc=nc,
        )
        return tuple(outs)

    devices = jax.devices()[:NCORES]
    assert len(devices) == NCORES
    mesh = Mesh(np.asarray(devices), ("core",))
    in_specs = (PartitionSpec("core"),) * (n_params + n_outs)
    out_specs = (PartitionSpec("core"),) * n_outs
    fn = jax.jit(
        shard_map(_body, mesh=mesh, in_specs=in_specs, out_specs=out_specs,
                  check_rep=False),
        donate_argnums=donate,
        keep_unused=True,
    )
    rt = {
        "fn": fn,
        "in_names": in_names,
        "out_names": out_names,
        "zero_shapes": zero_shapes,
        "n_params": n_params,
        "mesh": mesh,
        "nc": nc,
    }
    _RT["rt"] = rt
    return rt


def _make_masks():
    kk = np.arange(128, dtype=np.int64)[:, None]
    q = np.arange(512, dtype=np.int64)[None, :]
    masks = np.zeros((4, 128, 1024), dtype=np.float32)
    for m in range(4):
        half = ((128 * m + kk) <= q).astype(np.float32)
        masks[m, :, 0:512] = half
        masks[m, :, 512:1024] = half
    return masks


def _shard_inputs(query, key, value, Wq, bq, Wk, bk, Wv, bv, Wo, bo, pad_mask):
    f = np.float32
    import ml_dtypes
    bf = ml_dtypes.bfloat16
    query = np.asarray(query, f).reshape(B, S, D)
    key = np.asarray(key, f).reshape(B, S, D)
    value = np.asarray(value, f).reshape(B, S, D)
    consts = {
        "onescol": np.ones((1, 128), bf),
        "selp": np.ones((65, 64), f),
        "masks": _make_masks().astype(bf),
    }
    xT = {b: {
        "xqT": query[b].T.astype(bf),
        "xkT": key[b].T.astype(bf),
        "xvT": value[b].T.astype(bf),
    } for b in range(B)}
    wT = {
        "q": np.asarray(Wq, f).T,
        "k": np.asarray(Wk, f).T,
        "v": np.asarray(Wv, f).T,
        "o": np.asarray(Wo, f).T,
    }
    in_maps = []
    for c in range(NCORES):
        b = c // 2
        hh = c % 2
        sl = slice(DH * hh, DH * (hh + 1))
        m = {
            **xT[b],
            "wqt": wT["q"][:, sl].astype(bf),
            "wkt": wT["k"][:, sl].astype(bf),
            "wvt": wT["v"][:, sl].astype(bf),
            "wot": wT["o"][sl, :].astype(bf),
            "bq": np.ascontiguousarray(np.asarray(bq, f)[sl]),
            "bk": np.ascontiguousarray(np.asarray(bk, f)[sl]),
            "bv": np.asarray(bv, f)[sl].reshape(1, DH).astype(bf),
            **consts,
        }
        in_maps.append(m)
    return in_maps


def _run(rt, in_maps):
    import jax
    import numpy as np

    n = rt["n_params"]
    concat_in = [
        np.concatenate([np.asarray(in_maps[c][name]) for c in range(NCORES)], axis=0)
        for name in rt["in_names"]
    ]
    concat_zeros = [
        np.zeros((NCORES * sh[0], *sh[1:]), dt) for sh, dt in rt["zero_shapes"]
    ]
    out_arrs = rt["fn"](*concat_in, *concat_zeros)
    res = []
    for c in range(NCORES):
        d = {}
        for i, name in enumerate(rt["out_names"]):
            sh = rt["zero_shapes"][i][0]
            d[name] = np.asarray(out_arrs[i]).reshape(NCORES, *sh)[c]
        res.append(d)
    return res


def kernel(**inputs):
    rt = _get_runtime()
    in_maps = _shard_inputs(**inputs)
    res = _run(rt, in_maps)
    bo = np.asarray(inputs["bo"], np.float32)
    out = np.empty((B, S, D), dtype=np.float32)
    for b in range(B):
        out[b] = res[2 * b]["outp"] + res[2 * b + 1]["outp"] + bo
    return out
